# revision 16
# baseline (speedup 1.0000x reference)
"""BayesianGCN Trainium2 kernel (8 NeuronCores, SPMD).

Algorithm (per GCN layer): shard destination nodes across 8 cores.
  1. local linear z = h @ W.T (node-major, PE), z cast to bf16, DMA to DRAM
  2. AllGather z across the 8 cores (z_full in DRAM, bf16, 256B rows)
  3. per-edge gather of z_full[src] via SWDGE dma_gather (256B/edge) into
     edge-major SBUF tiles G (partition = edge)
  4. segment-sum via matmul-scatter: for each dst tile of 128 nodes,
     PSUM[dst, f] = sum_chunks S_chunk.T @ G_chunk, where S[e, dst] = w_e at
     column dst_e (built on DVE via (iota == dst)*w, one tensor_scalar per
     chunk).  The linear bias is folded in as a rank-1 matmul
     b.T(x)deg_w that doubles as the PSUM initializer (start=True).
  5. ReLU (ACT) -> node-major h tile in SBUF.
Dense tail (layers 4-6) runs feature-major on-chip; the final [2, 6250]
per-core output is transposed/concatenated on the host.

int16 gather indices only address 32768 rows, so edges are grouped by
source half (src < 25000 vs >= 25000); each (dst-tile, half) gets its own
edge chunks and each gather call covers one (supergroup-of-7-tiles, half).

Edge chunk geometry must be identical on all cores (one SPMD program), so
chunk counts per (tile, half) are the max over cores; slack slots are
padded with (idx=0, w=0) edges.
"""

import numpy as np
import ml_dtypes

import concourse.bacc as bacc
import concourse.mybir as mybir
import concourse.tile as tile
from concourse.bass_utils import run_bass_kernel_spmd
from concourse.library_config import mlp

BF16 = ml_dtypes.bfloat16

NCORES = 8
N = 50000
E = 800000
DIMS = [(128, 128), (128, 128), (128, 64), (64, 64), (64, 32), (32, 2)]

NPC = N // NCORES            # nodes per core
HALF = N // 2                # src-half boundary
TILE = 128
NTILES = (NPC + TILE - 1) // TILE
SGT = 7                      # dst tiles per supergroup (per gather call)
NSG = (NTILES + SGT - 1) // SGT
F32 = mybir.dt.float32
BF16D = mybir.dt.bfloat16
I16 = mybir.dt.int16


# ---------------------------------------------------------------- weights

def _bayes_weights(inputs):
    """Replicate the reference's reparameterized weight draws on host CPU."""
    import jax
    import jax.numpy as jnp

    cpu = jax.devices("cpu")[0]
    Ws, bs = [], []
    with jax.default_device(cpu):
        for l in range(1, 7):
            kw, kb = jax.random.split(jax.random.fold_in(jax.random.key(42), l))
            wmu = jnp.asarray(inputs[f"wmu{l}"])
            wrho = jnp.asarray(inputs[f"wrho{l}"])
            bmu = jnp.asarray(inputs[f"bmu{l}"])
            brho = jnp.asarray(inputs[f"brho{l}"])
            W = wmu + jax.nn.softplus(wrho) * jax.random.normal(kw, wmu.shape, jnp.float32)
            b = bmu + jax.nn.softplus(brho) * jax.random.normal(kb, bmu.shape, jnp.float32)
            Ws.append(np.asarray(W, np.float32))
            bs.append(np.asarray(b, np.float32))
    return Ws, bs


# ----------------------------------------------------------- preprocessing

class Geom:
    """Uniform (across cores) chunk geometry."""

    def __init__(self, nch):
        self.nch = nch                      # [NTILES][2] chunk counts
        self.sgs = [list(range(s, min(s + SGT, NTILES))) for s in range(0, NTILES, SGT)]
        # slot order: sg -> half -> tile -> chunk
        self.slot = {}                      # (t, h, ch) -> global slot id
        self.gslot = {}                     # (t, h, ch) -> slot offset within (sg, h)
        self.nslots_sg = {}                 # (sg, h) -> slots in that gather call
        self.idxcol = {}                    # (sg, h) -> int16-table column base
        s = 0
        col = 0
        for gi, ts in enumerate(self.sgs):
            for h in (0, 1):
                base = 0
                for t in ts:
                    for ch in range(nch[t][h]):
                        self.slot[(t, h, ch)] = s
                        self.gslot[(t, h, ch)] = base + ch
                        s += 1
                    base += nch[t][h]
                self.nslots_sg[(gi, h)] = base
                self.idxcol[(gi, h)] = col
                col += base * TILE // 16
        self.totslot = s
        self.totcol = col


def _preprocess(edge_index, edge_weight):
    src = np.ascontiguousarray(edge_index[0]).astype(np.int64)
    dst = np.ascontiguousarray(edge_index[1]).astype(np.int64)
    w = np.ascontiguousarray(edge_weight).astype(np.float32)

    core = dst // NPC
    dstl = dst % NPC
    tl = dstl // TILE
    half = (src >= HALF).astype(np.int64)
    idx16 = src - half * HALF

    key = (core * NTILES + tl) * 2 + half
    order = np.lexsort((src, key))
    skey = key[order]
    nkeys = NCORES * NTILES * 2
    lo = np.searchsorted(skey, np.arange(nkeys), side="left")
    hi = np.searchsorted(skey, np.arange(nkeys), side="right")
    cnt = (hi - lo).reshape(NCORES, NTILES, 2)

    nch = [[int(np.ceil(cnt[:, t, h].max() / TILE)) for h in (0, 1)] for t in range(NTILES)]
    g = Geom(nch)

    # weighted degree per node (for the folded bias)
    degw = np.bincount(dst, weights=w.astype(np.float64), minlength=N).astype(np.float32)

    per_core = []
    for c in range(NCORES):
        flat_idx = np.zeros(g.totslot * TILE, np.int16)
        flat_dl = np.zeros(g.totslot * TILE, np.float32)
        flat_w = np.zeros(g.totslot * TILE, np.float32)
        for gi, ts in enumerate(g.sgs):
            for h in (0, 1):
                for t in ts:
                    if g.nch[t][h] == 0:
                        continue
                    k = (c * NTILES + t) * 2 + h
                    seg = order[lo[k]:hi[k]]
                    n = len(seg)
                    s0 = g.slot[(t, h, 0)] * TILE
                    flat_idx[s0:s0 + n] = idx16[seg].astype(np.int16)
                    flat_dl[s0:s0 + n] = (dstl[seg] % TILE).astype(np.float32)
                    flat_w[s0:s0 + n] = w[seg]
        # idx table: per (sg, h) block, 16-partition wrap replicated 8x
        blocks = []
        for gi in range(len(g.sgs)):
            for h in (0, 1):
                ns = g.nslots_sg[(gi, h)]
                if ns == 0:
                    continue
                base = _block_base(g, gi, h) * TILE
                blk = flat_idx[base:base + ns * TILE]
                blocks.append(np.tile(blk.reshape(-1, 16).T, (8, 1)))
        idx_tab = np.concatenate(blocks, axis=1).astype(np.int16)
        per_core.append({
            "idx_tab": idx_tab,
            "dstloc": np.ascontiguousarray(flat_dl.reshape(g.totslot, TILE).T),
            "wtab": np.ascontiguousarray(flat_w.reshape(g.totslot, TILE).T),
            "degw": degw[c * NPC:(c + 1) * NPC].astype(BF16).reshape(1, NPC),
        })
    return g, per_core


def _block_base(g, gi, h):
    """Global slot id where the (sg, h) block starts."""
    for t in g.sgs[gi]:
        if g.nch[t][h] > 0:
            return g.slot[(t, h, 0)]
    return 0


# ------------------------------------------------------------- kernel build

_ABLATE = set()     # sim-attribution experiments: subsets of
                    # {"gather", "scatter", "sbuild", "ag"}


def _build(geom):
    AF = mybir.ActivationFunctionType
    ALU = mybir.AluOpType
    g = geom
    nc = bacc.Bacc("TRN2", num_devices=NCORES)

    # inputs
    xT_d = nc.dram_tensor("xT", [128, NPC], BF16D, kind="ExternalInput")
    idx_d = nc.dram_tensor("idx_tab", [128, g.totcol], I16, kind="ExternalInput")
    dl_d = nc.dram_tensor("dstloc", [128, g.totslot], F32, kind="ExternalInput")
    wt_d = nc.dram_tensor("wtab", [128, g.totslot], F32, kind="ExternalInput")
    degw_d = nc.dram_tensor("degw", [1, NPC], BF16D, kind="ExternalInput")
    iota_d = nc.dram_tensor("iota", [128, 128], BF16D, kind="ExternalInput")
    ident_d = nc.dram_tensor("ident", [128, 128], BF16D, kind="ExternalInput")
    WT_d = [nc.dram_tensor(f"WT{l}", [DIMS[l - 1][0], DIMS[l - 1][1]], BF16D,
                           kind="ExternalInput") for l in range(1, 7)]
    brow_d = [nc.dram_tensor(f"brow{l}", [1, DIMS[l - 1][1]], BF16D,
                             kind="ExternalInput") for l in range(1, 4)]
    bcol_d = [nc.dram_tensor(f"bcol{l}", [DIMS[l - 1][1], 1], F32,
                             kind="ExternalInput") for l in range(4, 7)]
    out_d = nc.dram_tensor("out", [2, NPC], F32, kind="ExternalOutput")

    gmax = max(g.nslots_sg.values()) if g.nslots_sg else 1
    ntile_last = NPC - (NTILES - 1) * TILE

    with tile.TileContext(nc) as tc:
        with (
            tc.tile_pool(name="const", bufs=1) as cpool,
            tc.tile_pool(name="gpool", bufs=3) as gpool,
            tc.tile_pool(name="spool", bufs=4) as spool,
            tc.tile_pool(name="hpool", bufs=2) as hpool,
            tc.tile_pool(name="work", bufs=3) as wpool,
            tc.tile_pool(name="ps_lin", bufs=2, space="PSUM") as ps_lin,
            tc.tile_pool(name="ps_tp", bufs=2, space="PSUM") as ps_tp,
            tc.tile_pool(name="ps_agg", bufs=2, space="PSUM") as ps_agg,
            tc.tile_pool(name="ps_tail", bufs=2, space="PSUM") as ps_tail,
            tc.tile_pool(name="dram", bufs=2, space="DRAM") as dpool,
        ):
            nc.gpsimd.load_library(mlp)

            # ---- resident constants
            xT = cpool.tile([128, NPC], BF16D, tag="xT")
            nc.sync.dma_start(xT[:], xT_d[:])
            idxt = cpool.tile([128, g.totcol], I16, tag="idx")
            nc.sync.dma_start(idxt[:], idx_d[:])
            dlt = cpool.tile([128, g.totslot], F32, tag="dl")
            nc.sync.dma_start(dlt[:], dl_d[:])
            wtt = cpool.tile([128, g.totslot], F32, tag="wt")
            nc.sync.dma_start(wtt[:], wt_d[:])
            degw = cpool.tile([1, NPC], BF16D, tag="degw")
            nc.sync.dma_start(degw[:], degw_d[:])
            iota = cpool.tile([128, 128], BF16D, tag="iota")
            nc.sync.dma_start(iota[:], iota_d[:])
            ident = cpool.tile([128, 128], BF16D, tag="ident")
            nc.sync.dma_start(ident[:], ident_d[:])
            WT = []
            for l in range(1, 7):
                t_ = cpool.tile([DIMS[l - 1][0], DIMS[l - 1][1]], BF16D, tag=f"WT{l}")
                nc.sync.dma_start(t_[:], WT_d[l - 1][:])
                WT.append(t_)
            brow = []
            for l in range(1, 4):
                t_ = cpool.tile([1, DIMS[l - 1][1]], BF16D, tag=f"brow{l}")
                nc.sync.dma_start(t_[:], brow_d[l - 1][:])
                brow.append(t_)
            bcol = []
            for l in range(4, 7):
                t_ = cpool.tile([DIMS[l - 1][1], 1], F32, tag=f"bcol{l}")
                nc.sync.dma_start(t_[:], bcol_d[l - 4][:])
                bcol.append(t_)

            h_prev = None           # node-major [128, NTILES*128] bf16
            for l in (1, 2, 3):
                fi, fo = DIMS[l - 1]
                # ---- 1) linear z = h @ W.T -> DRAM (node-major, bf16)
                agin = dpool.tile([NPC, 128], BF16D, tag="agin")
                for t in range(NTILES):
                    nt = ntile_last if t == NTILES - 1 else TILE
                    if l == 1:
                        lhsT = xT[:fi, t * TILE:t * TILE + nt]
                    else:
                        tp = ps_tp.tile([128, 128], BF16D, tag="tp")
                        nc.tensor.transpose(tp[:fi, :nt],
                                            h_prev[:nt, t * TILE:t * TILE + fi],
                                            ident[:nt, :nt])
                        hTt = wpool.tile([128, 128], BF16D, tag="hTt")
                        nc.scalar.activation(hTt[:fi, :nt], tp[:fi, :nt], AF.Copy)
                        lhsT = hTt[:fi, :nt]
                    zps = ps_lin.tile([128, 128], F32, tag="zps")
                    nc.tensor.matmul(zps[:nt, :fo], lhsT, WT[l - 1][:fi, :fo],
                                     start=True, stop=True)
                    zt = wpool.tile([128, 128], BF16D, tag="zt")
                    nc.scalar.activation(zt[:nt, :fo], zps[:nt, :fo], AF.Copy)
                    nc.sync.dma_start(agin[t * TILE:t * TILE + nt, :fo], zt[:nt, :fo])

                # ---- 2) AllGather
                agout = dpool.tile([N, 128], BF16D, tag="agout", addr_space="Shared")
                if "ag" not in _ABLATE:
                    nc.gpsimd.collective_compute(
                        "AllGather", mybir.AluOpType.bypass,
                        replica_groups=[list(range(NCORES))],
                        ins=[agin[:]], outs=[agout[:]],
                    )

                # ---- 3) gather + matmul-scatter per supergroup
                h_new = hpool.tile([128, NTILES * TILE], BF16D, tag="h")
                for gi, ts in enumerate(g.sgs):
                    Gt = {}
                    for h in (0, 1):
                        ns = g.nslots_sg[(gi, h)]
                        if ns == 0:
                            continue
                        Gt[h] = gpool.tile([128, gmax, 128], BF16D, tag="G",
                                           name=f"G_l{l}_sg{gi}_h{h}")
                        nidx = ns * TILE
                        c0 = g.idxcol[(gi, h)]
                        if "gather" not in _ABLATE:
                            nc.gpsimd.dma_gather(
                                Gt[h][:, :ns, :],
                                agout[h * HALF:(h + 1) * HALF, :],
                                idxt[:, c0:c0 + nidx // 16],
                                nidx, nidx, 128,
                                single_packet=False,
                            )
                    for t in ts:
                        nt = ntile_last if t == NTILES - 1 else TILE
                        nchunks = g.nch[t][0] + g.nch[t][1]
                        agg = ps_agg.tile([128, 128], F32, tag="agg")
                        t0 = t * TILE
                        nc.tensor.matmul(agg[:nt, :fo],
                                         degw[0:1, t0:t0 + nt], brow[l - 1][0:1, :fo],
                                         start=True, stop=(nchunks == 0))
                        done = 0
                        for h in (0, 1):
                            for ch in range(g.nch[t][h]):
                                s = g.slot[(t, h, ch)]
                                go = g.gslot[(t, h, ch)]
                                S = spool.tile([128, 128], BF16D, tag="S")
                                if "sbuild" not in _ABLATE:
                                    nc.vector.tensor_scalar(
                                        S[:], iota[:], dlt[:, s:s + 1], wtt[:, s:s + 1],
                                        op0=ALU.is_equal, op1=ALU.mult)
                                done += 1
                                if "scatter" not in _ABLATE:
                                    nc.tensor.matmul(agg[:nt, :fo],
                                                     S[:, :nt], Gt[h][:, go, :fo],
                                                     start=False, stop=(done == nchunks))
                        nc.scalar.activation(h_new[:nt, t * TILE:t * TILE + fo],
                                             agg[:nt, :fo], AF.Relu)
                h_prev = h_new

            # ---- dense tail, feature-major
            h3T = cpool.tile([64, NPC], BF16D, tag="h3T")
            for t in range(NTILES):
                nt = ntile_last if t == NTILES - 1 else TILE
                tp = ps_tp.tile([128, 128], BF16D, tag="tp")
                nc.tensor.transpose(tp[:64, :nt],
                                    h_prev[:nt, t * TILE:t * TILE + 64],
                                    ident[:nt, :nt])
                nc.scalar.activation(h3T[:64, t * TILE:t * TILE + nt], tp[:64, :nt], AF.Copy)

            hT = h3T
            for l in (4, 5, 6):
                fi, fo = DIMS[l - 1]
                last = l == 6
                dt_ = F32 if last else BF16D
                hTn = cpool.tile([fo, NPC], dt_, tag=f"h{l}T")
                for j0 in range(0, NPC, 512):
                    wj = min(512, NPC - j0)
                    ps = ps_tail.tile([64, 512], F32, tag="tail")
                    nc.tensor.matmul(ps[:fo, :wj], WT[l - 1][:fi, :fo],
                                     hT[:fi, j0:j0 + wj], start=True, stop=True)
                    nc.scalar.activation(hTn[:fo, j0:j0 + wj], ps[:fo, :wj],
                                         AF.Identity if last else AF.Relu,
                                         bias=bcol[l - 4][:fo, 0:1])
                hT = hTn
            nc.sync.dma_start(out_d[:], hT[:2, :])

    nc.compile()
    return nc


# ------------------------------------------------------------------ driver

_CACHE = {}


def _get_compiled(edge_index, edge_weight):
    key = (int(edge_index[0][:16].sum()), int(edge_index[1][:16].sum()), float(edge_weight[:16].sum()))
    if key not in _CACHE:
        geom, per_core = _preprocess(edge_index, edge_weight)
        nc = _build(geom)
        _CACHE[key] = (nc, geom, per_core)
    return _CACHE[key]


def kernel(x, edge_index, edge_weight, **kw):
    x = np.ascontiguousarray(x)
    edge_index = np.ascontiguousarray(edge_index)
    edge_weight = np.ascontiguousarray(edge_weight)
    nc, geom, per_core = _get_compiled(edge_index, edge_weight)
    Ws, bs = _bayes_weights(kw)

    iota = np.tile(np.arange(128, dtype=np.float32), (128, 1)).astype(BF16)
    ident = np.eye(128, dtype=np.float32).astype(BF16)

    in_maps = []
    for c in range(NCORES):
        m = dict(per_core[c])
        xs = x[c * NPC:(c + 1) * NPC].astype(np.float32)
        m["xT"] = np.ascontiguousarray(xs.T).astype(BF16)
        m["iota"] = iota
        m["ident"] = ident
        for l in range(1, 7):
            m[f"WT{l}"] = np.ascontiguousarray(Ws[l - 1].T).astype(BF16)
        for l in range(1, 4):
            m[f"brow{l}"] = bs[l - 1].reshape(1, -1).astype(BF16)
        for l in range(4, 7):
            m[f"bcol{l}"] = bs[l - 1].reshape(-1, 1).astype(np.float32)
        in_maps.append(m)

    import os
    trace = bool(int(os.environ.get("GCN_TRACE", "0")))
    kernel.last_in_maps = in_maps
    kernel.last_nc = nc
    res = run_bass_kernel_spmd(nc, in_maps, core_ids=list(range(NCORES)), trace=trace)
    kernel.last_result = res
    out = np.concatenate([res.results[c]["out"].T for c in range(NCORES)], axis=0)
    return out.astype(np.float32)
